# revision 19
# baseline (speedup 1.0000x reference)
"""BitSwarmLinear Trainium2 kernel (v4: 2D-sharded, nibble-packed population).

Computation (reference):
    swarm_sum = population.sum(axis=2)          # (out, in)
    w_eff     = sign(swarm_sum), sign(0) -> +1  # (out, in), +-1
    y         = einsum("bsi,oi->bso", x, w_eff) # (4, 4096, out)

Distribution (8 NeuronCores, 4-way out_features x 2-way tokens):
    core c: oc_shard = c % 4 (512 out rows), tok_shard = c // 4 (8192 tokens).
    Per-core HBM: x^T 33.5MB bf16 + pop 16.8MB nibble-packed + y 8.4MB bf16
    ~= 59MB << ~230us of tensor-engine work -> compute-bound (v1 shipped
    92MB/core and was DMA-bound at ~366us).

Host staging (lossless / layout-only):
    - population +-1.0 -> {0,1}; swarm pairs packed per byte (lo/hi nibble)
      -> 16 byte-planes. Bijective recode, no host arithmetic.
    - x -> bf16 x^T [128 in-part, 16 ko, 8192 tok].

Per-core device pipeline (one role per engine):
    - gpsimd: SWDGE pop-stream triggers (8 x 2MB tiles).
    - DVE: swarm reduce + binarize, 8 (q, half-K) units, all 2D contiguous
      APs. Planes 0-14 sum in u16 lanes (per-byte sums <= 15*17=255 -> no
      cross-byte carries), then exact fp32 nibble decode via magic-number
      floor: H = rne((S15+24.5)/16 + 2^23) - (2^23+2),
      count = S15 - 15H + (P15 - 15*[P15>=16]); w = 2*[count>=16]-1 (bf16).
    - PE: W^T stationary [128 in, 128 oc], x^T moving [128 in, 512 tok],
      PSUM [128 oc, 512 tok] over 16 K-tiles; 4 token-groups of 4 banks
      (PSUM fully double-buffered pass-to-pass). W stays SBUF-resident so
      x streams exactly once. Last oc-pass of each group runs bank-major
      so x-ring slots free early for the next group's chunks.
    - ACT ring: PSUM->SBUF bf16 casts + y stores.
    - sync ring: x chunk loads (16 x 512-token chunks, ring of 6).
"""

import os
import sys

import numpy as np

for _p in ("/root/.axon_site/_ro/trn_rl_repo", "/opt/trn_rl_repo"):
    if os.path.isdir(_p) and _p not in sys.path:
        sys.path.append(_p)

import ml_dtypes

try:
    import antenv.axon_hooks  # noqa: F401
except ImportError:
    try:
        import types as _types

        from trn_agent_boot.trn_boot import _ntff_profile_via_ctypes

        _hooks = _types.ModuleType("antenv.axon_hooks")
        _ntff_hook = _ntff_profile_via_ctypes("/opt/axon/libaxon_pjrt.so")
        _hooks.get_axon_ntff_profile_hook = lambda: _ntff_hook
        _hooks.set_axon_ntff_profile_hook = lambda h: None
        sys.modules["antenv.axon_hooks"] = _hooks
    except Exception:
        pass

import concourse.mybir as mybir
import concourse.tile as tile
from concourse import bacc
from concourse.bass_utils import run_bass_kernel_spmd

P = 128
IN_F = 2048
OUT_F = 2048
SWARM = 32
TOKENS = 4 * 4096
N_CORES = 8

OC_SHARDS = 4
TOK_SHARDS = 2
OUT_C = OUT_F // OC_SHARDS      # 512
TOK_C = TOKENS // TOK_SHARDS    # 8192

KO = IN_F // P           # 16 K-tiles
KBP = 2                  # half-K reduction units (8 K-tiles each)
KL = KO // KBP           # 8
JP = SWARM // 2          # 16 nibble byte-planes
Q = OUT_C // P           # 4
UW = KL * P              # 1024: unit width (8 k-tiles x 128 oc)

CHUNK = 512
N_CHUNKS = TOK_C // CHUNK        # 16
X_BUFS = 7
# group 0 is 6 banks wide (21.6us passes) so the DVE W-build waves (~27us
# per oc-block) hide behind it; later groups shrink to keep the x ring fed.
GROUPS = [list(range(0, 6)), list(range(6, 10)),
          list(range(10, 13)), list(range(13, 16))]

F32 = mybir.dt.float32
BF16 = mybir.dt.bfloat16
U8 = mybir.dt.uint8
U16 = mybir.dt.uint16

MAGIC = 8388608.0        # 2^23
ALU = mybir.AluOpType
ACTF = mybir.ActivationFunctionType


def _emit_unit(nc, pools, popt, w_u, q, kbp):
    """Swarm-reduce + binarize one (q, kbp) unit on the DVE (2D APs only).

    Writes into the unit's own W tile `w_u` [P, KL, P] so matmul gating is
    guaranteed per-(q, kbp) half-K regardless of sub-tile dep tracking.
    """
    acc_pool, s15_pool, tmp_pool = pools
    v = nc.vector
    acc16 = acc_pool.tile([P, UW // 2], U16, tag="acc", name=f"acc_{q}_{kbp}")
    v.tensor_add(acc16[:], popt[:, 0, :].bitcast(U16),
                 popt[:, 1, :].bitcast(U16))
    for j in range(2, JP - 1):
        v.tensor_add(acc16[:], acc16[:], popt[:, j, :].bitcast(U16))
    s15 = s15_pool.tile([P, UW], F32, tag="s15", name=f"s15_{q}_{kbp}")
    v.tensor_copy(out=s15[:], in_=acc16[:].bitcast(U8))
    # hh: (S15+24.5)/16, +2^23 (rounds to 2^23+2+H), then -> -15*H
    hh = tmp_pool.tile([P, UW], F32, tag="hh", name=f"hh_{q}_{kbp}")
    v.tensor_scalar(out=hh[:], in0=s15[:], scalar1=24.5, scalar2=0.0625,
                    op0=ALU.add, op1=ALU.mult)
    v.tensor_scalar(out=hh[:], in0=hh[:], scalar1=MAGIC, scalar2=None,
                    op0=ALU.add)
    v.tensor_scalar(out=hh[:], in0=hh[:], scalar1=MAGIC + 2.0,
                    scalar2=-15.0, op0=ALU.subtract, op1=ALU.mult)
    # e = P15 - 15*[P15>=16]  (= lo15+hi15 of plane 15)
    gg = tmp_pool.tile([P, UW], F32, tag="gg", name=f"gg_{q}_{kbp}")
    v.tensor_scalar(out=gg[:], in0=popt[:, JP - 1, :], scalar1=16,
                    scalar2=-15.0, op0=ALU.is_ge, op1=ALU.mult)
    pf = tmp_pool.tile([P, UW], F32, tag="pf", name=f"pf_{q}_{kbp}")
    v.tensor_copy(out=pf[:], in_=popt[:, JP - 1, :])
    v.tensor_add(pf[:], pf[:], gg[:])
    # count = S15 - 15H + e, then w0 = 2*[count>=16]
    cc = tmp_pool.tile([P, UW], F32, tag="cc", name=f"cc_{q}_{kbp}")
    v.tensor_add(cc[:], s15[:], hh[:])
    v.tensor_add(cc[:], cc[:], pf[:])
    v.tensor_scalar(out=cc[:], in0=cc[:], scalar1=16.0, scalar2=2.0,
                    op0=ALU.is_ge, op1=ALU.mult)
    # w = w0 - 1 into the bf16 W tile, one 2D write per k-tile
    for kl in range(KL):
        v.tensor_scalar(
            out=w_u[:, kl, :],
            in0=cc[:, kl * P:(kl + 1) * P],
            scalar1=1.0, scalar2=None, op0=ALU.subtract)


def build_nc():
    nc = bacc.Bacc(
        "TRN2",
        target_bir_lowering=False,
        debug=False,
        enable_asserts=False,
        num_devices=N_CORES,
    )

    xT = nc.dram_tensor("xT", [P, KO, TOK_C], BF16, kind="ExternalInput")
    pop = nc.dram_tensor("pop", [Q, KBP, P, JP, UW], U8, kind="ExternalInput")
    y = nc.dram_tensor("y", [Q, P, TOK_C], BF16, kind="ExternalOutput")

    xr = xT.ap()
    pr = pop.ap()
    yr = y.ap()

    with tile.TileContext(nc) as tc:
        with (
            tc.tile_pool(name="wsb", bufs=1) as w_pool,
            tc.tile_pool(name="xc", bufs=X_BUFS) as x_pool,
            tc.tile_pool(name="popt", bufs=2) as pop_pool,
            tc.tile_pool(name="acc", bufs=2) as acc_pool,
            tc.tile_pool(name="s15", bufs=2) as s15_pool,
            tc.tile_pool(name="tmp", bufs=1) as tmp_pool,
            tc.tile_pool(name="ys", bufs=2) as ys_pool,
            tc.tile_pool(name="psum", bufs=8, space="PSUM") as psum_pool,
        ):
            # one W tile per (q, kbp) unit -> matmuls gate on exactly the
            # half-K block they read, never on later units' writes
            w_tiles = {}
            for q in range(Q):
                for kbp in range(KBP):
                    w_tiles[(q, kbp)] = w_pool.tile(
                        [P, KL, P], BF16, tag=f"w{q}_{kbp}",
                        name=f"w{q}_{kbp}")
            pools = (acc_pool, s15_pool, tmp_pool)

            # ---- first x chunks (sync ring)
            xc_tiles = {}
            for c in range(X_BUFS):
                xc = x_pool.tile([P, KO, CHUNK], BF16, tag="xc",
                                 name=f"xc{c}")
                nc.sync.dma_start(xc[:], xr[:, :, c * CHUNK:(c + 1) * CHUNK])
                xc_tiles[c] = xc

            # ---- W build: pop stream on gpsimd SWDGE; reduce on DVE.
            # Each tile arrives as two half-DMAs so the plane sums can start
            # on the first half while the second streams.
            for q in range(Q):
                for kbp in range(KBP):
                    popt = pop_pool.tile([P, JP, UW], U8, tag="popt",
                                         name=f"popt_{q}_{kbp}")
                    nc.gpsimd.dma_start(popt[:, :JP // 2, :],
                                        pr[q, kbp, :, :JP // 2])
                    nc.gpsimd.dma_start(popt[:, JP // 2:, :],
                                        pr[q, kbp, :, JP // 2:])
                    _emit_unit(nc, pools, popt, w_tiles[(q, kbp)], q, kbp)

            # ---- remaining x chunks
            for c in range(X_BUFS, N_CHUNKS):
                xc = x_pool.tile([P, KO, CHUNK], BF16, tag="xc",
                                 name=f"xc{c}")
                nc.sync.dma_start(xc[:], xr[:, :, c * CHUNK:(c + 1) * CHUNK])
                xc_tiles[c] = xc

            # ---- matmul passes; casts + y stores on the ACT ring
            for g, chunks in enumerate(GROUPS):
                nb = len(chunks)
                for q in range(Q):
                    banks = [psum_pool.tile([P, CHUNK], F32, tag="ps",
                                            name=f"ps{g}_{q}_{b}")
                             for b in range(nb)]
                    if q < Q - 1:
                        for k in range(KO):
                            lhsT = w_tiles[(q, k // KL)][:, k % KL, :]
                            for b, c in enumerate(chunks):
                                nc.tensor.matmul(
                                    banks[b][:], lhsT, xc_tiles[c][:, k, :],
                                    start=(k == 0), stop=(k == KO - 1))
                        ys = ys_pool.tile([P, 6 * CHUNK], BF16, tag="ys",
                                          name=f"ys{g}_{q}")
                        for b, c in enumerate(chunks):
                            nc.scalar.activation(
                                out=ys[:, b * CHUNK:(b + 1) * CHUNK],
                                in_=banks[b][:], func=ACTF.Copy)
                    else:
                        # bank-major on the last oc-pass: each chunk's final
                        # read finishes early -> its x-ring slot frees for
                        # the next group's DMA; casts interleave per bank so
                        # the drain overlaps the remaining banks' matmuls.
                        ys = ys_pool.tile([P, 6 * CHUNK], BF16, tag="ys",
                                          name=f"ys{g}_{q}")
                        for b, c in enumerate(chunks):
                            for k in range(KO):
                                nc.tensor.matmul(
                                    banks[b][:],
                                    w_tiles[(q, k // KL)][:, k % KL, :],
                                    xc_tiles[c][:, k, :],
                                    start=(k == 0), stop=(k == KO - 1))
                            nc.scalar.activation(
                                out=ys[:, b * CHUNK:(b + 1) * CHUNK],
                                in_=banks[b][:], func=ACTF.Copy)
                    t0 = chunks[0] * CHUNK
                    nc.scalar.dma_start(
                        yr[q][:, t0:t0 + nb * CHUNK], ys[:, :nb * CHUNK])

    nc.compile()
    return nc


_NC_CACHE: dict = {}


def _get_nc():
    if "nc" not in _NC_CACHE:
        _NC_CACHE["nc"] = build_nc()
    return _NC_CACHE["nc"]


def stage_x_half(xf: np.ndarray, th: int):
    xh = np.ascontiguousarray(
        xf[th * TOK_C:(th + 1) * TOK_C].T
    ).astype(ml_dtypes.bfloat16)            # [in, tok]
    return np.ascontiguousarray(
        xh.reshape(KO, P, TOK_C).transpose(1, 0, 2))


def stage_pop_shard(pop_sl: np.ndarray):
    """pop slice [512, in, 32] (+-1.0) -> nibble [Q, KBP, P, JP, UW]."""
    pb = (pop_sl > 0).astype(np.uint8)
    pl = pb[..., 0::2] + 16 * pb[..., 1::2]                  # [512, in, JP]
    st = pl.reshape(Q, P, KBP, KL, P, JP)
    return np.ascontiguousarray(
        st.transpose(0, 2, 4, 5, 3, 1).reshape(Q, KBP, P, JP, UW))


def prep_inputs(x: np.ndarray, population: np.ndarray):
    xf = x.reshape(TOKENS, IN_F)
    x_halves = [stage_x_half(xf, th) for th in range(TOK_SHARDS)]
    pop_shards = [
        stage_pop_shard(population[o * OUT_C:(o + 1) * OUT_C])
        for o in range(OC_SHARDS)
    ]
    in_maps = []
    for c in range(N_CORES):
        ocs, th = c % OC_SHARDS, c // OC_SHARDS
        in_maps.append({"xT": x_halves[th], "pop": pop_shards[ocs]})
    return in_maps


def assemble(results):
    Y = np.empty((OUT_F, TOKENS), dtype=np.float32)
    for c, r in enumerate(results):
        ocs, th = c % OC_SHARDS, c // OC_SHARDS
        yc = r["y"].astype(np.float32).reshape(OUT_C, TOK_C)
        Y[ocs * OUT_C:(ocs + 1) * OUT_C, th * TOK_C:(th + 1) * TOK_C] = yc
    return np.ascontiguousarray(Y.T).reshape(4, TOKENS // 4, OUT_F)


def kernel(x: np.ndarray, population: np.ndarray):
    in_maps = prep_inputs(x, population)
    nc = _get_nc()
    res = run_bass_kernel_spmd(nc, in_maps, core_ids=list(range(N_CORES)))
    return assemble(res.results)


# revision 23
# speedup vs baseline: 1.0337x; 1.0337x over previous
"""BitSwarmLinear Trainium2 kernel (v4: 2D-sharded, nibble-packed population).

Computation (reference):
    swarm_sum = population.sum(axis=2)          # (out, in)
    w_eff     = sign(swarm_sum), sign(0) -> +1  # (out, in), +-1
    y         = einsum("bsi,oi->bso", x, w_eff) # (4, 4096, out)

Distribution (8 NeuronCores, 4-way out_features x 2-way tokens):
    core c: oc_shard = c % 4 (512 out rows), tok_shard = c // 4 (8192 tokens).
    Per-core HBM: x^T 33.5MB bf16 + pop 16.8MB nibble-packed + y 8.4MB bf16
    ~= 59MB << ~230us of tensor-engine work -> compute-bound (v1 shipped
    92MB/core and was DMA-bound at ~366us).

Host staging (lossless / layout-only):
    - population +-1.0 -> {0,1}; swarm pairs packed per byte (lo/hi nibble)
      -> 16 byte-planes. Bijective recode, no host arithmetic.
    - x -> bf16 x^T [128 in-part, 16 ko, 8192 tok].

Per-core device pipeline (one role per engine):
    - gpsimd: SWDGE pop-stream triggers (8 x 2MB tiles).
    - DVE: swarm reduce + binarize, 8 (q, half-K) units, all 2D contiguous
      APs. Planes 0-14 sum in u16 lanes (per-byte sums <= 15*17=255 -> no
      cross-byte carries), then exact fp32 nibble decode via magic-number
      floor: H = rne((S15+24.5)/16 + 2^23) - (2^23+2),
      count = S15 - 15H + (P15 - 15*[P15>=16]); w = 2*[count>=16]-1 (bf16).
    - PE: W^T stationary [128 in, 128 oc], x^T moving [128 in, 512 tok],
      PSUM [128 oc, 512 tok] over 16 K-tiles; 4 token-groups of 4 banks
      (PSUM fully double-buffered pass-to-pass). W stays SBUF-resident so
      x streams exactly once. Last oc-pass of each group runs bank-major
      so x-ring slots free early for the next group's chunks.
    - ACT ring: PSUM->SBUF bf16 casts + y stores.
    - sync ring: x chunk loads (16 x 512-token chunks, ring of 6).
"""

import os
import sys

import numpy as np

for _p in ("/root/.axon_site/_ro/trn_rl_repo", "/opt/trn_rl_repo"):
    if os.path.isdir(_p) and _p not in sys.path:
        sys.path.append(_p)

import ml_dtypes

try:
    import antenv.axon_hooks  # noqa: F401
except ImportError:
    try:
        import types as _types

        from trn_agent_boot.trn_boot import _ntff_profile_via_ctypes

        _hooks = _types.ModuleType("antenv.axon_hooks")
        _ntff_hook = _ntff_profile_via_ctypes("/opt/axon/libaxon_pjrt.so")
        _hooks.get_axon_ntff_profile_hook = lambda: _ntff_hook
        _hooks.set_axon_ntff_profile_hook = lambda h: None
        sys.modules["antenv.axon_hooks"] = _hooks
    except Exception:
        pass

import concourse.mybir as mybir
import concourse.tile as tile
from concourse import bacc
from concourse.bass_utils import run_bass_kernel_spmd

P = 128
IN_F = 2048
OUT_F = 2048
SWARM = 32
TOKENS = 4 * 4096
N_CORES = 8

OC_SHARDS = 4
TOK_SHARDS = 2
OUT_C = OUT_F // OC_SHARDS      # 512
TOK_C = TOKENS // TOK_SHARDS    # 8192

KO = IN_F // P           # 16 K-tiles
KBP = 2                  # half-K reduction units (8 K-tiles each)
KL = KO // KBP           # 8
JP = SWARM // 2          # 16 nibble byte-planes
Q = OUT_C // P           # 4
UW = KL * P              # 1024: unit width (8 k-tiles x 128 oc)

CHUNK = 512
N_CHUNKS = TOK_C // CHUNK        # 16
X_BUFS = 7
GROUPS = [list(range(4 * g, 4 * g + 4)) for g in range(4)]

F32 = mybir.dt.float32
BF16 = mybir.dt.bfloat16
U8 = mybir.dt.uint8
U16 = mybir.dt.uint16

MAGIC = 8388608.0        # 2^23
ALU = mybir.AluOpType
ACTF = mybir.ActivationFunctionType


def _emit_unit(nc, pools, popt, w_u, q, kbp):
    """Swarm-reduce + binarize one (q, kbp) unit on the DVE (2D APs only).

    Writes into the unit's own W tile `w_u` [P, KL, P] so matmul gating is
    guaranteed per-(q, kbp) half-K regardless of sub-tile dep tracking.
    """
    acc_pool, s15_pool, tmp_pool = pools
    v = nc.vector
    acc16 = acc_pool.tile([P, UW // 2], U16, tag="acc", name=f"acc_{q}_{kbp}")
    v.tensor_add(acc16[:], popt[:, 0, :].bitcast(U16),
                 popt[:, 1, :].bitcast(U16))
    for j in range(2, JP - 1):
        v.tensor_add(acc16[:], acc16[:], popt[:, j, :].bitcast(U16))
    s15 = s15_pool.tile([P, UW], F32, tag="s15", name=f"s15_{q}_{kbp}")
    v.tensor_copy(out=s15[:], in_=acc16[:].bitcast(U8))
    # hh: (S15+24.5)/16, +2^23 (rounds to 2^23+2+H), then -> -15*H
    hh = tmp_pool.tile([P, UW], F32, tag="hh", name=f"hh_{q}_{kbp}")
    v.tensor_scalar(out=hh[:], in0=s15[:], scalar1=24.5, scalar2=0.0625,
                    op0=ALU.add, op1=ALU.mult)
    v.tensor_scalar(out=hh[:], in0=hh[:], scalar1=MAGIC, scalar2=None,
                    op0=ALU.add)
    v.tensor_scalar(out=hh[:], in0=hh[:], scalar1=MAGIC + 2.0,
                    scalar2=-15.0, op0=ALU.subtract, op1=ALU.mult)
    # e = P15 - 15*[P15>=16]  (= lo15+hi15 of plane 15)
    gg = tmp_pool.tile([P, UW], F32, tag="gg", name=f"gg_{q}_{kbp}")
    v.tensor_scalar(out=gg[:], in0=popt[:, JP - 1, :], scalar1=16,
                    scalar2=-15.0, op0=ALU.is_ge, op1=ALU.mult)
    pf = tmp_pool.tile([P, UW], F32, tag="pf", name=f"pf_{q}_{kbp}")
    v.tensor_copy(out=pf[:], in_=popt[:, JP - 1, :])
    v.tensor_add(pf[:], pf[:], gg[:])
    # count = S15 - 15H + e, then w0 = 2*[count>=16]
    cc = tmp_pool.tile([P, UW], F32, tag="cc", name=f"cc_{q}_{kbp}")
    v.tensor_add(cc[:], s15[:], hh[:])
    v.tensor_add(cc[:], cc[:], pf[:])
    v.tensor_scalar(out=cc[:], in0=cc[:], scalar1=16.0, scalar2=2.0,
                    op0=ALU.is_ge, op1=ALU.mult)
    # w = w0 - 1 into the bf16 W tile: one wide 2D write (matmuls slice
    # per-k columns out of the 2D tile directly)
    v.tensor_scalar(out=w_u[:], in0=cc[:], scalar1=1.0, scalar2=None,
                    op0=ALU.subtract)


def build_nc():
    nc = bacc.Bacc(
        "TRN2",
        target_bir_lowering=False,
        debug=False,
        enable_asserts=False,
        num_devices=N_CORES,
    )

    xT = nc.dram_tensor("xT", [P, KO, TOK_C], BF16, kind="ExternalInput")
    pop = nc.dram_tensor("pop", [Q, KBP, P, JP, UW], U8, kind="ExternalInput")
    y = nc.dram_tensor("y", [Q, P, TOK_C], BF16, kind="ExternalOutput")

    xr = xT.ap()
    pr = pop.ap()
    yr = y.ap()

    with tile.TileContext(nc) as tc:
        with (
            tc.tile_pool(name="wsb", bufs=1) as w_pool,
            tc.tile_pool(name="xc", bufs=X_BUFS) as x_pool,
            tc.tile_pool(name="popt", bufs=2) as pop_pool,
            tc.tile_pool(name="acc", bufs=2) as acc_pool,
            tc.tile_pool(name="s15", bufs=2) as s15_pool,
            tc.tile_pool(name="tmp", bufs=1) as tmp_pool,
            tc.tile_pool(name="ys", bufs=2) as ys_pool,
            tc.tile_pool(name="psum", bufs=8, space="PSUM") as psum_pool,
        ):
            # one W tile per (q, kbp) unit -> matmuls gate on exactly the
            # half-K block they read, never on later units' writes
            w_tiles = {}
            for q in range(Q):
                for kbp in range(KBP):
                    w_tiles[(q, kbp)] = w_pool.tile(
                        [P, UW], BF16, tag=f"w{q}_{kbp}",
                        name=f"w{q}_{kbp}")
            pools = (acc_pool, s15_pool, tmp_pool)

            # ---- first x chunks (sync ring)
            xc_tiles = {}
            for c in range(X_BUFS):
                xc = x_pool.tile([P, KO, CHUNK], BF16, tag="xc",
                                 name=f"xc{c}")
                nc.sync.dma_start(xc[:], xr[:, :, c * CHUNK:(c + 1) * CHUNK])
                xc_tiles[c] = xc

            # ---- W build: pop stream on gpsimd SWDGE; reduce on DVE.
            # Each tile arrives as two half-DMAs so the plane sums can start
            # on the first half while the second streams.
            for q in range(Q):
                for kbp in range(KBP):
                    popt = pop_pool.tile([P, JP, UW], U8, tag="popt",
                                         name=f"popt_{q}_{kbp}")
                    nc.gpsimd.dma_start(popt[:, :JP // 2, :],
                                        pr[q, kbp, :, :JP // 2])
                    nc.gpsimd.dma_start(popt[:, JP // 2:, :],
                                        pr[q, kbp, :, JP // 2:])
                    _emit_unit(nc, pools, popt, w_tiles[(q, kbp)], q, kbp)

            # ---- remaining x chunks
            for c in range(X_BUFS, N_CHUNKS):
                xc = x_pool.tile([P, KO, CHUNK], BF16, tag="xc",
                                 name=f"xc{c}")
                nc.sync.dma_start(xc[:], xr[:, :, c * CHUNK:(c + 1) * CHUNK])
                xc_tiles[c] = xc

            # ---- matmul passes; casts + y stores on the ACT ring
            for g, chunks in enumerate(GROUPS):
                nb = len(chunks)
                for q in range(Q):
                    banks = [psum_pool.tile([P, CHUNK], F32, tag="ps",
                                            name=f"ps{g}_{q}_{b}")
                             for b in range(nb)]
                    if q < Q - 1:
                        for k in range(KO):
                            lhsT = w_tiles[(q, k // KL)][:, (k % KL) * P:(k % KL + 1) * P]
                            for b, c in enumerate(chunks):
                                nc.tensor.matmul(
                                    banks[b][:], lhsT, xc_tiles[c][:, k, :],
                                    start=(k == 0), stop=(k == KO - 1))
                    else:
                        # bank-major on the last oc-pass: each chunk's final
                        # read finishes early -> its x-ring slot frees for
                        # the next group's DMA.
                        for b, c in enumerate(chunks):
                            for k in range(KO):
                                nc.tensor.matmul(
                                    banks[b][:],
                                    w_tiles[(q, k // KL)][:, (k % KL) * P:(k % KL + 1) * P],
                                    xc_tiles[c][:, k, :],
                                    start=(k == 0), stop=(k == KO - 1))
                    ys = ys_pool.tile([P, 4 * CHUNK], BF16, tag="ys",
                                      name=f"ys{g}_{q}")
                    for b, c in enumerate(chunks):
                        nc.scalar.activation(
                            out=ys[:, b * CHUNK:(b + 1) * CHUNK],
                            in_=banks[b][:], func=ACTF.Copy)
                    t0 = chunks[0] * CHUNK
                    nc.scalar.dma_start(
                        yr[q][:, t0:t0 + nb * CHUNK], ys[:, :nb * CHUNK])

    nc.compile()
    return nc


_NC_CACHE: dict = {}


def _get_nc():
    if "nc" not in _NC_CACHE:
        _NC_CACHE["nc"] = build_nc()
    return _NC_CACHE["nc"]


def stage_x_half(xf: np.ndarray, th: int):
    xh = np.ascontiguousarray(
        xf[th * TOK_C:(th + 1) * TOK_C].T
    ).astype(ml_dtypes.bfloat16)            # [in, tok]
    return np.ascontiguousarray(
        xh.reshape(KO, P, TOK_C).transpose(1, 0, 2))


def stage_pop_shard(pop_sl: np.ndarray):
    """pop slice [512, in, 32] (+-1.0) -> nibble [Q, KBP, P, JP, UW]."""
    pb = (pop_sl > 0).astype(np.uint8)
    pl = pb[..., 0::2] + 16 * pb[..., 1::2]                  # [512, in, JP]
    st = pl.reshape(Q, P, KBP, KL, P, JP)
    return np.ascontiguousarray(
        st.transpose(0, 2, 4, 5, 3, 1).reshape(Q, KBP, P, JP, UW))


def prep_inputs(x: np.ndarray, population: np.ndarray):
    xf = x.reshape(TOKENS, IN_F)
    x_halves = [stage_x_half(xf, th) for th in range(TOK_SHARDS)]
    pop_shards = [
        stage_pop_shard(population[o * OUT_C:(o + 1) * OUT_C])
        for o in range(OC_SHARDS)
    ]
    in_maps = []
    for c in range(N_CORES):
        ocs, th = c % OC_SHARDS, c // OC_SHARDS
        in_maps.append({"xT": x_halves[th], "pop": pop_shards[ocs]})
    return in_maps


def assemble(results):
    Y = np.empty((OUT_F, TOKENS), dtype=np.float32)
    for c, r in enumerate(results):
        ocs, th = c % OC_SHARDS, c // OC_SHARDS
        yc = r["y"].astype(np.float32).reshape(OUT_C, TOK_C)
        Y[ocs * OUT_C:(ocs + 1) * OUT_C, th * TOK_C:(th + 1) * TOK_C] = yc
    return np.ascontiguousarray(Y.T).reshape(4, TOKENS // 4, OUT_F)


def kernel(x: np.ndarray, population: np.ndarray):
    in_maps = prep_inputs(x, population)
    nc = _get_nc()
    res = run_bass_kernel_spmd(nc, in_maps, core_ids=list(range(N_CORES)))
    return assemble(res.results)
